# revision 41
# baseline (speedup 1.0000x reference)
"""Expert-parallel MoE FFN (ChronosMOEFeedForward) for 8 Trainium2 cores.

Strategy (sharding_hint: expert-parallel):
  - Router (softmax + top-2 over E=16 experts) computed on host in fp32 —
    top-k decisions must match the fp32 reference's ordering, and the router
    GEMM is ~0.1% of total FLOPs.
  - The 16 experts are sharded 2-per-core across 8 cores. Tokens routed to
    each expert are gathered on host (the "all-to-all dispatch"), padded to a
    fixed capacity C, and shipped transposed as [H, C] so the device GEMM
    chain needs no on-device transposes.
  - Per core the device computes, per expert e:
        gT = Wg[e].T @ XeT           [I, C]   (bf16 inputs, fp32 PSUM accum)
        uT = Wu[e].T @ XeT           [I, C]
        aT = silu(gT) * uT           [I, C]   (Silu on Act engine, one DVE mul)
        yT = Wd[e].T @ aT            [H, C]   (tokens stay on the free dim)
  - The top-2 combine weight is applied on the HOST during the scatter —
    it is linear in y (w*(a@wd) == (w*a)@wd), so the device ships the raw
    expert output and the host multiplies rows by the fp32 combine weight.
    This removes the weight DMA, the PE broadcast outer-product, and one
    DVE multiply per eviction group, and is exact fp32.
  - Host scatters each expert's y rows back to the owning tokens ("combine").
    A token's two expert contributions land in two disjoint slot arrays
    (top-1 slot, top-2 slot), so the combine is collision-free fancy
    indexing plus one add — no np.add.at.

Performance notes (all tuned by interleaved loop-slope A/B on the 8 cores;
single-queue / single-core intuitions did not survive contact with HW):
  - Input DMA descriptors batch 2 k-tiles each (KB=2) with the DRAM side
    viewed as (partition, k, col) via AP.rearrange so element pairing
    matches the SBUF layout; this halves the ~0.5-0.9 us fixed
    per-descriptor DGE processing cost. y outputs batch 2 m-tiles (YB=2).
  - Descriptors are spread over the two independent DGE paths (HWDGE via
    the sync queue, software DGE via gpsimd); the scalar queue shares the
    HWDGE with sync, so it only carries y. xg rides sync entirely, weights
    alternate sync/gpsimd (policy "v4").
  - Phase A runs chunk-rounds spanning all 8 m-tiles (8 PSUM banks): the
    PE then consumes weight k-tiles no faster than the DMA engines deliver
    them, so the first expert's phase A does not starve. Phase B keeps
    narrow 2-m-tile rounds (wider was neutral-to-worse on HW).
  - The LIGHT expert (cap <= 512) runs first: its wide rounds are the
    slowest consumers, and the heavy expert's weights prefetch behind them.
  - wg/wu share a 3-slot pool: the next expert's weights start streaming
    a full phase earlier than with 2 slots (-5% measured, robustness
    against transient DGE stalls); xg is double-buffered.

The dense reference formulation computes all 16 experts for every token;
routed top-2 computes only 2 — an 8x FLOP reduction, plus bf16 matmuls with
fp32 PSUM accumulation.
"""

import numpy as np
import ml_dtypes

import concourse.mybir as mybir
import concourse.tile as tile
from concourse import bacc
from concourse.bass_utils import run_bass_kernel_spmd

# Problem shapes (hardcoded per contract).
H = 2048        # hidden size
I = 1024        # moe intermediate size
E = 16          # num experts
TOPK = 2
B, S = 4, 1024
T = B * S       # 4096 tokens
N_CORES = 8
EPC = E // N_CORES  # experts per core = 2
# Per-slot token capacities: each core gets one "heavy" expert (slot 0) and
# one "light" expert (slot 1). The host assigns the 8 heaviest-loaded experts
# to slot 0. Mean load is 512 +- ~22, so the light half of the experts almost
# always fits in 512; the few tokens that overflow fall back to exact numpy.
CAPS = (640, 512)
C = CAPS[0]      # max capacity (DRAM params are padded to this)

BF16 = ml_dtypes.bfloat16

KT_H = H // 128  # 16 k-tiles over H
MT_I = I // 128  # 8 m-tiles over I
KT_I = I // 128  # 8 k-tiles over I

_CACHE = {}

# DMA queue assignment policy for input tensors, selected by hardware A/B.
#   v1: xg/gw alternate sync<->gpsimd per k; uw alternates; wd alternates.
#   v2: xg sync, gw scalar, uw gpsimd, wd alternates sync/gpsimd.
#   v3: all inputs round-robin (sync, gpsimd, scalar) by a global counter.
_QPOLICY = "v4"

# Phase-A round structure: "chunk" = all 8 m-tiles per round (8 PSUM banks,
# per-k consumption ~927 ns, matches DMA delivery); "pair" = 2 m-tiles x all
# chunks (4 banks, faster per-k consumption but starves on expert 0).
_ROUNDS = "chunk"

# Phase-B round width in m-tiles: "pair" = 2 m-tiles per round (4 banks for
# the 556-cap slot, 2 for the 512 slot); "wide" = as many m-tiles as fill
# all 8 PSUM banks.
_BROUNDS = "pair"

# DMA descriptor batching: k-tiles per input descriptor (1 or 2) and
# m-tiles per y output descriptor (1 or 2). Fewer descriptors amortize the
# ~460 ns fixed per-descriptor processing cost on the DGE engines.
_KB = 2
_YB = 2
_KBX = None  # xg k-batch override; None = same as _KB
_WBUFS = 3  # weight pool slots: one extra expert of prefetch lookahead (-5% measured)
_DMA_COLS = None  # diagnostic: clamp DMA transfer widths (timing modules only)


def _spans(kt, kb, first_small=False):
    """Descriptor spans (start, width) covering kt k-tiles in chunks of kb.

    first_small=True emits the first k-tile as its own descriptor so the
    first dependent matmul waits on ~1/kb of the data.
    """
    if not first_small or kb <= 1:
        return [(ko, min(kb, kt - ko)) for ko in range(0, kt, kb)]
    out = [(0, 1)]
    ko = 1
    while ko < kt:
        w = min(kb, kt - ko)
        out.append((ko, w))
        ko += w
    return out


def _queues(nc):
    ctr = [0]

    def pick(tensor, ko):
        if _QPOLICY == "v1":
            if tensor == "y":
                return nc.scalar
            if tensor == "xg":
                return nc.sync if ko % 2 == 0 else nc.gpsimd
            if tensor == "gw":
                return nc.gpsimd if ko % 2 == 0 else nc.sync
            if tensor == "uw":
                return nc.sync if ko % 2 == 0 else nc.gpsimd
            return nc.gpsimd if ko % 2 == 0 else nc.sync
        if _QPOLICY in ("v4", "v7"):
            # ALL weight descriptors ride the gpsimd/software-DGE path while
            # the hardware DGE (sync+scalar) is dedicated to the
            # latency-sensitive xg loads and y stores. Measured faster than
            # per-descriptor alternation (v8/v9) by ~5%: the PE's k-pacing
            # then never waits on the busier engine, and the xg/y service
            # is never disrupted by bulk weight traffic.
            if tensor == "y":
                return nc.gpsimd if _QPOLICY == "v7" else nc.scalar
            if tensor == "xg":
                return nc.sync
            return nc.gpsimd
        if _QPOLICY in ("v8", "v9"):
            # True per-descriptor alternation under KB=2 (v4's ko%2 test is
            # always even there, so v4 sends ALL weights to the SWDGE).
            # v8: weights alternate gpsimd-first; v9: gw starts on the
            # faster HWDGE so the first expert's first weights land sooner.
            if tensor == "y":
                return nc.scalar
            if tensor == "xg":
                return nc.sync
            d = (ko // _KB) % 2
            if _QPOLICY == "v9" and tensor == "gw":
                return nc.sync if d == 0 else nc.gpsimd
            return nc.gpsimd if d == 0 else nc.sync
        if _QPOLICY == "v6":
            # 2:1 weight skew toward HWDGE, y on gpsimd.
            if tensor == "y":
                return nc.gpsimd
            if tensor == "xg":
                return nc.sync
            return nc.gpsimd if ko % 3 == 0 else nc.sync
        if _QPOLICY == "v2":
            if tensor == "xg":
                return nc.sync
            if tensor == "gw":
                return nc.scalar
            if tensor == "uw":
                return nc.gpsimd
            return nc.sync if ko % 2 == 0 else nc.gpsimd
        # v3: global round-robin over all three queues
        q = (nc.sync, nc.gpsimd, nc.scalar)[ctr[0] % 3]
        ctr[0] += 1
        return q

    return pick


def _build_nc(caps=CAPS, loop_r=None, internal=False):
    """Build the per-core Bass module (SPMD: all cores run this program).

    caps: per-slot token capacities. kernel() derives them from the actual
    routing (phase A streams tokens on the matmul free dim, so capacity is
    not 128-quantized there) and caches one compiled module per caps value.
    loop_r/internal are for the timing harness only: Internal DRAM I/O (no
    host transfers) with the body repeated loop_r times on-device.
    """
    import contextlib

    nc = bacc.Bacc(None, target_bir_lowering=False)
    f32 = mybir.dt.float32
    bf16 = mybir.dt.bfloat16

    if internal:
        xg = nc.dram_tensor("xg", [EPC, H, C], bf16)
        gww = nc.dram_tensor("gww", [EPC, H, I], bf16)
        uww = nc.dram_tensor("uww", [EPC, H, I], bf16)
        wdp = nc.dram_tensor("wdp", [EPC, I, H], bf16)
        y = nc.dram_tensor("y", [EPC, H, C], bf16)
        done = nc.declare_dram_parameter("done", [1, 1], f32, isOutput=True)
    else:
        xg = nc.declare_dram_parameter("xg", [EPC, H, C], bf16, isOutput=False)
        gww = nc.declare_dram_parameter("gww", [EPC, H, I], bf16, isOutput=False)
        uww = nc.declare_dram_parameter("uww", [EPC, H, I], bf16, isOutput=False)
        wdp = nc.declare_dram_parameter("wdp", [EPC, I, H], bf16, isOutput=False)
        y = nc.declare_dram_parameter("y", [EPC, H, C], bf16, isOutput=True)
        done = None

    with tile.TileContext(nc) as tc:
        with (
            tc.tile_pool(name="wpool", bufs=_WBUFS) as wpool,  # wg/wu share slots
            tc.tile_pool(name="xpool", bufs=2) as xpool,
            tc.tile_pool(name="wdpool", bufs=1) as wdpool,
            tc.tile_pool(name="apool", bufs=1) as apool,
            tc.tile_pool(name="small", bufs=2) as small,
            tc.tile_pool(name="yp", bufs=4) as yp,
            tc.tile_pool(name="ps", bufs=8, space="PSUM") as ps,
        ):
            loop_cm = (
                tc.For_i(0, loop_r, 1) if loop_r else contextlib.nullcontext()
            )
            with loop_cm:
                _emit_body(nc, tc, caps, xg, gww, uww, wdp, y,
                           wpool, xpool, wdpool, apool, small, yp, ps)

            if internal:
                dn = small.tile([1, 1], f32, tag="done")
                nc.any.memset(dn[:], 1.0)
                nc.sync.dma_start(out=done[:], in_=dn[:])

    nc.compile()
    return nc


def _emit_body(nc, tc, caps, xg, gww, uww, wdp, y,
               wpool, xpool, wdpool, apool, small, yp, ps):
    f32 = mybir.dt.float32
    bf16 = mybir.dt.bfloat16
    for e in range(EPC):
        Ce = caps[e]
        # one free-dim chunk if it fits a PSUM bank, else an even split
        NCH = Ce if Ce <= 512 else (Ce + 1) // 2
        # DMA issue order = need order. All inputs stream on the sync (SP)
        # queue in k-paced (xg, gw) pairs; y output traffic rides the Act
        # queue so it never head-of-line blocks the next expert's prefetch.
        # Input DMAs are spread over multiple DMA queues: a single queue
        # sustains only ~650-900 ns per 128-row descriptor, which is within
        # ~20% of the PE time and starves phase A of the first expert.
        # Queue policy (see _QPOLICY) is chosen by hardware A/B.
        qs = _queues(nc)
        kb = _KB
        kbx = _KBX if _KBX is not None else _KB
        dcw = _DMA_COLS
        cex = min(dcw, Ce) if dcw else Ce
        ci = min(dcw, I) if dcw else I
        ch = min(dcw, H) if dcw else H
        gw_sb = wpool.tile([128, KT_H, I], bf16, tag="guw")
        xg_sb = xpool.tile([128, KT_H, C], bf16, tag="xg")
        # Batched descriptors cover kb k-tiles each; the DRAM side is viewed
        # as (p, k, c) so DRAM row k*128+p pairs with SBUF partition p,
        # k-slot k (the DMA pairs source/dest elements in AP iteration
        # order, and the SBUF side iterates partition-major).
        # NOTE: emitting the first expert's first k-tile as its own smaller
        # descriptor was tried (first_small=True) and is sim-neutral: the
        # ~1 us earlier start is paid back by mis-aligned later spans.
        for ko, kw in _spans(KT_H, kbx):
            qs("xg", ko).dma_start(
                out=xg_sb[:, ko : ko + kw, :cex],
                in_=xg[e, ko * 128 : (ko + kw) * 128, :cex].rearrange(
                    "(k p) c -> p k c", p=128
                ),
            )
        for ko, kw in _spans(KT_H, kb):
            # NOTE: routing the first gw descriptor via the scalar queue
            # (HWDGE) to shorten single-shot startup was tried and measured
            # WORSE in sim (+1.5 us head): scalar shares the HWDGE with
            # sync and queues behind the xg descriptor + Act dispatches.
            qs("gw", ko).dma_start(
                out=gw_sb[:, ko : ko + kw, :ci],
                in_=gww[e, ko * 128 : (ko + kw) * 128, :ci].rearrange(
                    "(k p) c -> p k c", p=128
                ),
            )
        uw_sb = wpool.tile([128, KT_H, I], bf16, tag="guw")
        for ko in range(0, KT_H, kb):
            qs("uw", ko).dma_start(
                out=uw_sb[:, ko : ko + kb, :ci],
                in_=uww[e, ko * 128 : (ko + kb) * 128, :ci].rearrange(
                    "(k p) c -> p k c", p=128
                ),
            )
        wd_sb = wdpool.tile([128, KT_I, H], bf16, tag="wd")
        for ko in range(0, KT_I, kb):
            qs("wd", ko).dma_start(
                out=wd_sb[:, ko : ko + kb, :ch],
                in_=wdp[e, ko * 128 : (ko + kb) * 128, :ch].rearrange(
                    "(k p) c -> p k c", p=128
                ),
            )

        sg_sb = apool.tile([128, MT_I, C], bf16, tag="sg")
        a_sb = apool.tile([128, MT_I, C], bf16, tag="a")

        # ---- phase A: gT/uT in chunk-rounds spanning ALL 8 m-tiles (8 PSUM
        # banks wide). Wide rounds slow the per-k-tile consumption rate to
        # ~927 ns (8 m x 278 rows), matching the 2-engine DMA delivery pace
        # (~650-914 ns per k-tile pair) so the first expert's phase A never
        # starves the PE waiting on weight k-tiles.
        for mat in range(2):  # 0: g (silu), 1: u (mul)
            w_sb = gw_sb if mat == 0 else uw_sb
            if _ROUNDS == "chunk":
                rounds = [
                    [(m, c0, min(NCH, Ce - c0)) for m in range(MT_I)]
                    for c0 in range(0, Ce, NCH)
                ]
            else:  # pair: 2 m-tiles x all chunks per round, 4 banks wide
                rounds = [
                    [
                        (m, c0, min(NCH, Ce - c0))
                        for m in range(half * 2, half * 2 + 2)
                        for c0 in range(0, Ce, NCH)
                    ]
                    for half in range(4)
                ]
            for groups in rounds:
                psts = {}
                for m, c0, w in groups:
                    psts[(m, c0)] = ps.tile(
                        [128, 512], f32, tag="ps", name=f"ps_{m}_{c0}"
                    )
                for k in range(KT_H):
                    for m, c0, w in groups:
                        nc.tensor.matmul(
                            psts[(m, c0)][:, :w],
                            lhsT=w_sb[:, k, m * 128 : (m + 1) * 128],
                            rhs=xg_sb[:, k, c0 : c0 + w],
                            start=(k == 0),
                            stop=(k == KT_H - 1),
                        )
                for m, c0, w in groups:
                    pt = psts[(m, c0)][:, :w]
                    if mat == 0:
                        # silu(g) on the Act engine, straight from PSUM
                        nc.scalar.activation(
                            sg_sb[:, m, c0 : c0 + w], pt,
                            mybir.ActivationFunctionType.Silu,
                        )
                    else:
                        nc.vector.tensor_mul(
                            a_sb[:, m, c0 : c0 + w],
                            sg_sb[:, m, c0 : c0 + w],
                            pt,
                        )

        # ---- phase B: yT = Wd.T @ a   [H, Ce] — tokens stay on the
        # free dim, so only the exact Ce columns are streamed (no
        # 128-row quantization like the y = a.T @ Wd layout)
        bchunks = (
            [(0, Ce)] if Ce <= 512 else [(0, NCH), (NCH, Ce - NCH)]
        )
        bmw = 2 if _BROUNDS == "pair" else (8 // len(bchunks))
        for pair in range(H // (128 * bmw)):  # m-tile groups over H
            bgroups = [
                (m, c0, w)
                for m in range(pair * bmw, pair * bmw + bmw)
                for c0, w in bchunks
            ]
            bpsts = {}
            for m, c0, w in bgroups:
                bpsts[(m, c0)] = ps.tile(
                    [128, 512], f32, tag="ps", name=f"bps_{m}_{c0}"
                )
            for k in range(KT_I):
                for m, c0, w in bgroups:
                    nc.tensor.matmul(
                        bpsts[(m, c0)][:, :w],
                        lhsT=wd_sb[:, k, m * 128 : (m + 1) * 128],
                        rhs=a_sb[:, k, c0 : c0 + w],
                        start=(k == 0),
                        stop=(k == KT_I - 1),
                    )
            ypt = yp.tile([128, bmw, C], bf16, tag="ysb", name=f"yt_{pair}")
            for m, c0, w in bgroups:
                nc.vector.tensor_copy(
                    ypt[:, m - pair * bmw, c0 : c0 + w], bpsts[(m, c0)][:, :w]
                )
            if _YB == 1:
                for j in range(bmw):
                    m = pair * bmw + j
                    qs("y", m).dma_start(
                        out=y[e, m * 128 : (m + 1) * 128, :cex],
                        in_=ypt[:, j, :cex],
                    )
            else:
                qs("y", pair).dma_start(
                    out=y[
                        e, pair * bmw * 128 : (pair + 1) * bmw * 128, :cex
                    ].rearrange("(m p) c -> p m c", p=128),
                    in_=ypt[:, :, :cex],
                )


def _route(xf, gate_w):
    """Top-2 routing, mirroring the fp32 reference semantics exactly."""
    logits = xf @ gate_w.T.astype(np.float32)          # [T, E]
    logits -= logits.max(axis=-1, keepdims=True)
    scores = np.exp(logits)
    scores /= scores.sum(axis=-1, keepdims=True)
    i1 = scores.argmax(axis=-1)
    s1 = scores[np.arange(T), i1]
    masked = scores.copy()
    masked[np.arange(T), i1] = -np.inf
    i2 = masked.argmax(axis=-1)
    s2 = scores[np.arange(T), i2]
    denom = s1 + s2 + 1e-20
    return i1, s1 / denom, i2, s2 / denom


def _expert_np(xrows, wts, wg_e, wu_e, wd_e):
    """Exact fp32 fallback for capacity-overflow tokens (rare)."""
    g = xrows @ wg_e
    u = xrows @ wu_e
    a = (g / (1.0 + np.exp(-g))) * u * wts[:, None]
    return a @ wd_e


def _pack(xf, gate_w, wg, wu, wd):
    """Route + gather + pack per-core device inputs.

    Experts are assigned to (core, slot) by load: the 8 heaviest go to the
    slot-0 capacity, the 8 lightest to the slot-1 capacity. The assignment is
    pure host-side data placement — the SPMD program is identical on every
    core.
    """
    i1, w1, i2, w2 = _route(xf, gate_w)
    per_e = []
    for e in range(E):
        l1 = np.nonzero(i1 == e)[0]
        l2 = np.nonzero(i2 == e)[0]
        toks = np.concatenate([l1, l2])
        wts = np.concatenate([w1[l1], w2[l2]])
        ranks = np.concatenate(
            [np.zeros(len(l1), np.int8), np.ones(len(l2), np.int8)]
        )
        per_e.append((toks, ranks, wts))
    loads = [len(pe[0]) for pe in per_e]
    order = np.argsort([-n for n in loads], kind="stable")
    # exact capacities from this routing. The LIGHT expert goes in slot 0:
    # its wide phase-A rounds consume k-tiles slower than the DMA delivers
    # them, and the heavy expert's weights prefetch during its ~80 us of
    # compute, so neither expert's phase A ever starves the PE. Slot 0 stays
    # at <=512 (one phase-A chunk); slot 1 covers the heaviest expert up to
    # the DRAM padding; rare overflow tokens go to the exact numpy path.
    caps = (
        min(max(loads[order[N_CORES]], 128), CAPS[1]),
        min(loads[order[0]], CAPS[0]),
    )

    in_maps = []
    tok_lists = []
    for c in range(N_CORES):
        xgc = np.zeros((EPC, H, C), BF16)
        core_toks = []
        experts = [int(order[2 * N_CORES - 1 - c]), int(order[c])]
        for j in range(EPC):
            e = experts[j]
            toks, ranks, wts = per_e[e]
            n_dev = min(len(toks), caps[j])
            xgc[j, :, :n_dev] = xf[toks[:n_dev]].T.astype(BF16)
            core_toks.append((toks, ranks, wts, n_dev, e))
        tok_lists.append(core_toks)
        in_maps.append(
            {
                "xg": xgc,
                "gww": wg[experts].astype(BF16),
                "uww": wu[experts].astype(BF16),
                "wdp": wd[experts].astype(BF16),
            }
        )
    return in_maps, tok_lists, caps


def kernel(x, gate_w, wg, wu, wd):
    in_dtype = x.dtype
    xf = np.ascontiguousarray(np.asarray(x).reshape(T, H), dtype=np.float32)
    gate_w = np.asarray(gate_w, dtype=np.float32)
    wg = np.asarray(wg, dtype=np.float32)
    wu = np.asarray(wu, dtype=np.float32)
    wd = np.asarray(wd, dtype=np.float32)

    in_maps, tok_lists, caps = _pack(xf, gate_w, wg, wu, wd)
    if caps not in _CACHE:
        _CACHE[caps] = _build_nc(caps)
    nc = _CACHE[caps]
    out1 = np.zeros((T, H), np.float32)
    out2 = np.zeros((T, H), np.float32)

    res = run_bass_kernel_spmd(nc, in_maps, core_ids=list(range(N_CORES)))
    _CACHE["last_in_maps"] = in_maps
    _CACHE["last_caps"] = caps
    _CACHE["nc"] = nc

    for c in range(N_CORES):
        yc = res.results[c]["y"]                           # [EPC, H, C] bf16
        for j in range(EPC):
            toks, ranks, wts, n_dev, e = tok_lists[c][j]
            # raw expert output; combine weight applied here in fp32
            yr = yc[j, :, :n_dev].T.astype(np.float32) * wts[:n_dev, None]
            sel1 = ranks[:n_dev] == 0
            sel2 = ~sel1
            out1[toks[:n_dev][sel1]] = yr[sel1]
            out2[toks[:n_dev][sel2]] = yr[sel2]
            if len(toks) > n_dev:                          # capacity overflow
                extra = toks[n_dev:]
                yextra = _expert_np(xf[extra], wts[n_dev:], wg[e], wu[e], wd[e])
                r = ranks[n_dev:]
                out1[extra[r == 0]] = yextra[r == 0]
                out2[extra[r == 1]] = yextra[r == 1]

    out = (out1 + out2).reshape(B, S, H)
    return out.astype(in_dtype, copy=False)



# revision 42
# speedup vs baseline: 1.0224x; 1.0224x over previous
"""Expert-parallel MoE FFN (ChronosMOEFeedForward) for 8 Trainium2 cores.

Strategy (sharding_hint: expert-parallel):
  - Router (softmax + top-2 over E=16 experts) computed on host in fp32 —
    top-k decisions must match the fp32 reference's ordering, and the router
    GEMM is ~0.1% of total FLOPs.
  - The 16 experts are sharded 2-per-core across 8 cores. Tokens routed to
    each expert are gathered on host (the "all-to-all dispatch"), padded to a
    fixed capacity C, and shipped transposed as [H, C] so the device GEMM
    chain needs no on-device transposes.
  - Per core the device computes, per expert e:
        gT = Wg[e].T @ XeT           [I, C]   (bf16 inputs, fp32 PSUM accum)
        uT = Wu[e].T @ XeT           [I, C]
        aT = silu(gT) * uT           [I, C]   (Silu on Act engine, one DVE mul)
        yT = Wd[e].T @ aT            [H, C]   (tokens stay on the free dim)
  - The top-2 combine weight is applied on the HOST during the scatter —
    it is linear in y (w*(a@wd) == (w*a)@wd), so the device ships the raw
    expert output and the host multiplies rows by the fp32 combine weight.
    This removes the weight DMA, the PE broadcast outer-product, and one
    DVE multiply per eviction group, and is exact fp32.
  - Host scatters each expert's y rows back to the owning tokens ("combine").
    A token's two expert contributions land in two disjoint slot arrays
    (top-1 slot, top-2 slot), so the combine is collision-free fancy
    indexing plus one add — no np.add.at.

Performance notes (all tuned by interleaved loop-slope A/B on the 8 cores;
single-queue / single-core intuitions did not survive contact with HW):
  - Input DMA descriptors batch 2 k-tiles each (KB=2) with the DRAM side
    viewed as (partition, k, col) via AP.rearrange so element pairing
    matches the SBUF layout; this halves the ~0.5-0.9 us fixed
    per-descriptor DGE processing cost. y outputs batch 2 m-tiles (YB=2).
  - Descriptors are spread over the two independent DGE paths (HWDGE via
    the sync queue, software DGE via gpsimd); the scalar queue shares the
    HWDGE with sync, so it only carries y. xg rides sync entirely, weights
    alternate sync/gpsimd (policy "v4").
  - Phase A runs chunk-rounds spanning all 8 m-tiles (8 PSUM banks): the
    PE then consumes weight k-tiles no faster than the DMA engines deliver
    them, so the first expert's phase A does not starve. Phase B keeps
    narrow 2-m-tile rounds (wider was neutral-to-worse on HW).
  - The LIGHT expert (cap <= 512) runs first: its wide rounds are the
    slowest consumers, and the heavy expert's weights prefetch behind them.
  - wg/wu share a 3-slot pool: the next expert's weights start streaming
    a full phase earlier than with 2 slots (-5% measured, robustness
    against transient DGE stalls); xg is double-buffered.

The dense reference formulation computes all 16 experts for every token;
routed top-2 computes only 2 — an 8x FLOP reduction, plus bf16 matmuls with
fp32 PSUM accumulation.
"""

import numpy as np
import ml_dtypes

import concourse.mybir as mybir
import concourse.tile as tile
from concourse import bacc
from concourse.bass_utils import run_bass_kernel_spmd

# Problem shapes (hardcoded per contract).
H = 2048        # hidden size
I = 1024        # moe intermediate size
E = 16          # num experts
TOPK = 2
B, S = 4, 1024
T = B * S       # 4096 tokens
N_CORES = 8
EPC = E // N_CORES  # experts per core = 2
# Per-slot token capacities: each core gets one "heavy" expert (slot 0) and
# one "light" expert (slot 1). The host assigns the 8 heaviest-loaded experts
# to slot 0. Mean load is 512 +- ~22, so the light half of the experts almost
# always fits in 512; the few tokens that overflow fall back to exact numpy.
CAPS = (640, 512)
C = CAPS[0]      # max capacity (DRAM params are padded to this)

BF16 = ml_dtypes.bfloat16

KT_H = H // 128  # 16 k-tiles over H
MT_I = I // 128  # 8 m-tiles over I
KT_I = I // 128  # 8 k-tiles over I

_CACHE = {}

# DMA queue assignment policy for input tensors, selected by hardware A/B.
#   v1: xg/gw alternate sync<->gpsimd per k; uw alternates; wd alternates.
#   v2: xg sync, gw scalar, uw gpsimd, wd alternates sync/gpsimd.
#   v3: all inputs round-robin (sync, gpsimd, scalar) by a global counter.
_QPOLICY = "v4"

# Phase-A round structure: "chunk" = all 8 m-tiles per round (8 PSUM banks,
# per-k consumption ~927 ns, matches DMA delivery); "pair" = 2 m-tiles x all
# chunks (4 banks, faster per-k consumption but starves on expert 0).
_ROUNDS = "chunk"

# Phase-B round width in m-tiles: "pair" = 2 m-tiles per round (4 banks for
# the 556-cap slot, 2 for the 512 slot); "wide" = as many m-tiles as fill
# all 8 PSUM banks.
_BROUNDS = "pair"

# DMA descriptor batching: k-tiles per input descriptor (1 or 2) and
# m-tiles per y output descriptor (1 or 2). Fewer descriptors amortize the
# ~460 ns fixed per-descriptor processing cost on the DGE engines.
_KB = 2
_YB = 2
_KBX = None  # xg k-batch override; None = same as _KB
_WBUFS = 3  # weight pool slots: one extra expert of prefetch lookahead (-5% measured)
_WDBUFS = 1  # wd pool slots (2 overflows SBUF when _WBUFS=3)
_DMA_COLS = None  # diagnostic: clamp DMA transfer widths (timing modules only)


def _spans(kt, kb, first_small=False):
    """Descriptor spans (start, width) covering kt k-tiles in chunks of kb.

    first_small=True emits the first k-tile as its own descriptor so the
    first dependent matmul waits on ~1/kb of the data.
    """
    if not first_small or kb <= 1:
        return [(ko, min(kb, kt - ko)) for ko in range(0, kt, kb)]
    out = [(0, 1)]
    ko = 1
    while ko < kt:
        w = min(kb, kt - ko)
        out.append((ko, w))
        ko += w
    return out


def _queues(nc):
    ctr = [0]

    def pick(tensor, ko):
        if _QPOLICY == "v1":
            if tensor == "y":
                return nc.scalar
            if tensor == "xg":
                return nc.sync if ko % 2 == 0 else nc.gpsimd
            if tensor == "gw":
                return nc.gpsimd if ko % 2 == 0 else nc.sync
            if tensor == "uw":
                return nc.sync if ko % 2 == 0 else nc.gpsimd
            return nc.gpsimd if ko % 2 == 0 else nc.sync
        if _QPOLICY in ("v4", "v7"):
            # ALL weight descriptors ride the gpsimd/software-DGE path while
            # the hardware DGE (sync+scalar) is dedicated to the
            # latency-sensitive xg loads and y stores. Measured faster than
            # per-descriptor alternation (v8/v9) by ~5%: the PE's k-pacing
            # then never waits on the busier engine, and the xg/y service
            # is never disrupted by bulk weight traffic.
            if tensor == "y":
                return nc.gpsimd if _QPOLICY == "v7" else nc.scalar
            if tensor == "xg":
                return nc.sync
            return nc.gpsimd
        if _QPOLICY in ("v8", "v9"):
            # True per-descriptor alternation under KB=2 (v4's ko%2 test is
            # always even there, so v4 sends ALL weights to the SWDGE).
            # v8: weights alternate gpsimd-first; v9: gw starts on the
            # faster HWDGE so the first expert's first weights land sooner.
            if tensor == "y":
                return nc.scalar
            if tensor == "xg":
                return nc.sync
            d = (ko // _KB) % 2
            if _QPOLICY == "v9" and tensor == "gw":
                return nc.sync if d == 0 else nc.gpsimd
            return nc.gpsimd if d == 0 else nc.sync
        if _QPOLICY == "v6":
            # 2:1 weight skew toward HWDGE, y on gpsimd.
            if tensor == "y":
                return nc.gpsimd
            if tensor == "xg":
                return nc.sync
            return nc.gpsimd if ko % 3 == 0 else nc.sync
        if _QPOLICY == "v2":
            if tensor == "xg":
                return nc.sync
            if tensor == "gw":
                return nc.scalar
            if tensor == "uw":
                return nc.gpsimd
            return nc.sync if ko % 2 == 0 else nc.gpsimd
        # v3: global round-robin over all three queues
        q = (nc.sync, nc.gpsimd, nc.scalar)[ctr[0] % 3]
        ctr[0] += 1
        return q

    return pick


def _build_nc(caps=CAPS, loop_r=None, internal=False):
    """Build the per-core Bass module (SPMD: all cores run this program).

    caps: per-slot token capacities. kernel() derives them from the actual
    routing (phase A streams tokens on the matmul free dim, so capacity is
    not 128-quantized there) and caches one compiled module per caps value.
    loop_r/internal are for the timing harness only: Internal DRAM I/O (no
    host transfers) with the body repeated loop_r times on-device.
    """
    import contextlib

    nc = bacc.Bacc(None, target_bir_lowering=False)
    f32 = mybir.dt.float32
    bf16 = mybir.dt.bfloat16

    if internal:
        xg = nc.dram_tensor("xg", [EPC, H, C], bf16)
        gww = nc.dram_tensor("gww", [EPC, H, I], bf16)
        uww = nc.dram_tensor("uww", [EPC, H, I], bf16)
        wdp = nc.dram_tensor("wdp", [EPC, I, H], bf16)
        y = nc.dram_tensor("y", [EPC, H, C], bf16)
        done = nc.declare_dram_parameter("done", [1, 1], f32, isOutput=True)
    else:
        xg = nc.declare_dram_parameter("xg", [EPC, H, C], bf16, isOutput=False)
        gww = nc.declare_dram_parameter("gww", [EPC, H, I], bf16, isOutput=False)
        uww = nc.declare_dram_parameter("uww", [EPC, H, I], bf16, isOutput=False)
        wdp = nc.declare_dram_parameter("wdp", [EPC, I, H], bf16, isOutput=False)
        y = nc.declare_dram_parameter("y", [EPC, H, C], bf16, isOutput=True)
        done = None

    with tile.TileContext(nc) as tc:
        with (
            tc.tile_pool(name="wpool", bufs=_WBUFS) as wpool,  # wg/wu share slots
            tc.tile_pool(name="xpool", bufs=2) as xpool,
            tc.tile_pool(name="wdpool", bufs=_WDBUFS) as wdpool,
            tc.tile_pool(name="apool", bufs=1) as apool,
            tc.tile_pool(name="small", bufs=2) as small,
            tc.tile_pool(name="yp", bufs=4) as yp,
            tc.tile_pool(name="ps", bufs=8, space="PSUM") as ps,
        ):
            loop_cm = (
                tc.For_i(0, loop_r, 1) if loop_r else contextlib.nullcontext()
            )
            with loop_cm:
                _emit_body(nc, tc, caps, xg, gww, uww, wdp, y,
                           wpool, xpool, wdpool, apool, small, yp, ps)

            if internal:
                dn = small.tile([1, 1], f32, tag="done")
                nc.any.memset(dn[:], 1.0)
                nc.sync.dma_start(out=done[:], in_=dn[:])

    nc.compile()
    return nc


def _emit_body(nc, tc, caps, xg, gww, uww, wdp, y,
               wpool, xpool, wdpool, apool, small, yp, ps):
    f32 = mybir.dt.float32
    bf16 = mybir.dt.bfloat16
    for e in range(EPC):
        Ce = caps[e]
        # one free-dim chunk if it fits a PSUM bank, else an even split
        NCH = Ce if Ce <= 512 else (Ce + 1) // 2
        # DMA issue order = need order. All inputs stream on the sync (SP)
        # queue in k-paced (xg, gw) pairs; y output traffic rides the Act
        # queue so it never head-of-line blocks the next expert's prefetch.
        # Input DMAs are spread over multiple DMA queues: a single queue
        # sustains only ~650-900 ns per 128-row descriptor, which is within
        # ~20% of the PE time and starves phase A of the first expert.
        # Queue policy (see _QPOLICY) is chosen by hardware A/B.
        qs = _queues(nc)
        kb = _KB
        kbx = _KBX if _KBX is not None else _KB
        dcw = _DMA_COLS
        cex = min(dcw, Ce) if dcw else Ce
        ci = min(dcw, I) if dcw else I
        ch = min(dcw, H) if dcw else H
        gw_sb = wpool.tile([128, KT_H, I], bf16, tag="guw")
        xg_sb = xpool.tile([128, KT_H, C], bf16, tag="xg")
        # Batched descriptors cover kb k-tiles each; the DRAM side is viewed
        # as (p, k, c) so DRAM row k*128+p pairs with SBUF partition p,
        # k-slot k (the DMA pairs source/dest elements in AP iteration
        # order, and the SBUF side iterates partition-major).
        # NOTE: emitting the first expert's first k-tile as its own smaller
        # descriptor was tried (first_small=True) and is sim-neutral: the
        # ~1 us earlier start is paid back by mis-aligned later spans.
        for ko, kw in _spans(KT_H, kbx):
            qs("xg", ko).dma_start(
                out=xg_sb[:, ko : ko + kw, :cex],
                in_=xg[e, ko * 128 : (ko + kw) * 128, :cex].rearrange(
                    "(k p) c -> p k c", p=128
                ),
            )
        for ko, kw in _spans(KT_H, kb):
            # NOTE: routing the first gw descriptor via the scalar queue
            # (HWDGE) to shorten single-shot startup was tried and measured
            # WORSE in sim (+1.5 us head): scalar shares the HWDGE with
            # sync and queues behind the xg descriptor + Act dispatches.
            qs("gw", ko).dma_start(
                out=gw_sb[:, ko : ko + kw, :ci],
                in_=gww[e, ko * 128 : (ko + kw) * 128, :ci].rearrange(
                    "(k p) c -> p k c", p=128
                ),
            )
        uw_sb = wpool.tile([128, KT_H, I], bf16, tag="guw")
        for ko in range(0, KT_H, kb):
            qs("uw", ko).dma_start(
                out=uw_sb[:, ko : ko + kb, :ci],
                in_=uww[e, ko * 128 : (ko + kb) * 128, :ci].rearrange(
                    "(k p) c -> p k c", p=128
                ),
            )
        wd_sb = wdpool.tile([128, KT_I, H], bf16, tag="wd")
        for ko in range(0, KT_I, kb):
            qs("wd", ko).dma_start(
                out=wd_sb[:, ko : ko + kb, :ch],
                in_=wdp[e, ko * 128 : (ko + kb) * 128, :ch].rearrange(
                    "(k p) c -> p k c", p=128
                ),
            )

        sg_sb = apool.tile([128, MT_I, C], bf16, tag="sg")
        a_sb = apool.tile([128, MT_I, C], bf16, tag="a")

        # ---- phase A: gT/uT in chunk-rounds spanning ALL 8 m-tiles (8 PSUM
        # banks wide). Wide rounds slow the per-k-tile consumption rate to
        # ~927 ns (8 m x 278 rows), matching the 2-engine DMA delivery pace
        # (~650-914 ns per k-tile pair) so the first expert's phase A never
        # starves the PE waiting on weight k-tiles.
        for mat in range(2):  # 0: g (silu), 1: u (mul)
            w_sb = gw_sb if mat == 0 else uw_sb
            if _ROUNDS == "chunk":
                rounds = [
                    [(m, c0, min(NCH, Ce - c0)) for m in range(MT_I)]
                    for c0 in range(0, Ce, NCH)
                ]
            else:  # pair: 2 m-tiles x all chunks per round, 4 banks wide
                rounds = [
                    [
                        (m, c0, min(NCH, Ce - c0))
                        for m in range(half * 2, half * 2 + 2)
                        for c0 in range(0, Ce, NCH)
                    ]
                    for half in range(4)
                ]
            for groups in rounds:
                psts = {}
                for m, c0, w in groups:
                    psts[(m, c0)] = ps.tile(
                        [128, 512], f32, tag="ps", name=f"ps_{m}_{c0}"
                    )
                for k in range(KT_H):
                    for m, c0, w in groups:
                        nc.tensor.matmul(
                            psts[(m, c0)][:, :w],
                            lhsT=w_sb[:, k, m * 128 : (m + 1) * 128],
                            rhs=xg_sb[:, k, c0 : c0 + w],
                            start=(k == 0),
                            stop=(k == KT_H - 1),
                        )
                for m, c0, w in groups:
                    pt = psts[(m, c0)][:, :w]
                    if mat == 0:
                        # silu(g) on the Act engine, straight from PSUM
                        nc.scalar.activation(
                            sg_sb[:, m, c0 : c0 + w], pt,
                            mybir.ActivationFunctionType.Silu,
                        )
                    else:
                        nc.vector.tensor_mul(
                            a_sb[:, m, c0 : c0 + w],
                            sg_sb[:, m, c0 : c0 + w],
                            pt,
                        )

        # ---- phase B: yT = Wd.T @ a   [H, Ce] — tokens stay on the
        # free dim, so only the exact Ce columns are streamed (no
        # 128-row quantization like the y = a.T @ Wd layout)
        bchunks = (
            [(0, Ce)] if Ce <= 512 else [(0, NCH), (NCH, Ce - NCH)]
        )
        bmw = 2 if _BROUNDS == "pair" else (8 // len(bchunks))
        for pair in range(H // (128 * bmw)):  # m-tile groups over H
            bgroups = [
                (m, c0, w)
                for m in range(pair * bmw, pair * bmw + bmw)
                for c0, w in bchunks
            ]
            bpsts = {}
            for m, c0, w in bgroups:
                bpsts[(m, c0)] = ps.tile(
                    [128, 512], f32, tag="ps", name=f"bps_{m}_{c0}"
                )
            for k in range(KT_I):
                for m, c0, w in bgroups:
                    nc.tensor.matmul(
                        bpsts[(m, c0)][:, :w],
                        lhsT=wd_sb[:, k, m * 128 : (m + 1) * 128],
                        rhs=a_sb[:, k, c0 : c0 + w],
                        start=(k == 0),
                        stop=(k == KT_I - 1),
                    )
            ypt = yp.tile([128, bmw, C], bf16, tag="ysb", name=f"yt_{pair}")
            for m, c0, w in bgroups:
                nc.vector.tensor_copy(
                    ypt[:, m - pair * bmw, c0 : c0 + w], bpsts[(m, c0)][:, :w]
                )
            if _YB == 1:
                for j in range(bmw):
                    m = pair * bmw + j
                    qs("y", m).dma_start(
                        out=y[e, m * 128 : (m + 1) * 128, :cex],
                        in_=ypt[:, j, :cex],
                    )
            else:
                qs("y", pair).dma_start(
                    out=y[
                        e, pair * bmw * 128 : (pair + 1) * bmw * 128, :cex
                    ].rearrange("(m p) c -> p m c", p=128),
                    in_=ypt[:, :, :cex],
                )


def _route(xf, gate_w):
    """Top-2 routing, mirroring the fp32 reference semantics exactly."""
    logits = xf @ gate_w.T.astype(np.float32)          # [T, E]
    logits -= logits.max(axis=-1, keepdims=True)
    scores = np.exp(logits)
    scores /= scores.sum(axis=-1, keepdims=True)
    i1 = scores.argmax(axis=-1)
    s1 = scores[np.arange(T), i1]
    masked = scores.copy()
    masked[np.arange(T), i1] = -np.inf
    i2 = masked.argmax(axis=-1)
    s2 = scores[np.arange(T), i2]
    denom = s1 + s2 + 1e-20
    return i1, s1 / denom, i2, s2 / denom


def _expert_np(xrows, wts, wg_e, wu_e, wd_e):
    """Exact fp32 fallback for capacity-overflow tokens (rare)."""
    g = xrows @ wg_e
    u = xrows @ wu_e
    a = (g / (1.0 + np.exp(-g))) * u * wts[:, None]
    return a @ wd_e


def _pack(xf, gate_w, wg, wu, wd):
    """Route + gather + pack per-core device inputs.

    Experts are assigned to (core, slot) by load: the 8 heaviest go to the
    slot-0 capacity, the 8 lightest to the slot-1 capacity. The assignment is
    pure host-side data placement — the SPMD program is identical on every
    core.
    """
    i1, w1, i2, w2 = _route(xf, gate_w)
    per_e = []
    for e in range(E):
        l1 = np.nonzero(i1 == e)[0]
        l2 = np.nonzero(i2 == e)[0]
        toks = np.concatenate([l1, l2])
        wts = np.concatenate([w1[l1], w2[l2]])
        ranks = np.concatenate(
            [np.zeros(len(l1), np.int8), np.ones(len(l2), np.int8)]
        )
        per_e.append((toks, ranks, wts))
    loads = [len(pe[0]) for pe in per_e]
    order = np.argsort([-n for n in loads], kind="stable")
    # exact capacities from this routing. The LIGHT expert goes in slot 0:
    # its wide phase-A rounds consume k-tiles slower than the DMA delivers
    # them, and the heavy expert's weights prefetch during its ~80 us of
    # compute, so neither expert's phase A ever starves the PE. Slot 0 stays
    # at <=512 (one phase-A chunk); slot 1 covers the heaviest expert up to
    # the DRAM padding; rare overflow tokens go to the exact numpy path.
    caps = (
        min(max(loads[order[N_CORES]], 128), CAPS[1]),
        min(loads[order[0]], CAPS[0]),
    )

    in_maps = []
    tok_lists = []
    for c in range(N_CORES):
        xgc = np.zeros((EPC, H, C), BF16)
        core_toks = []
        experts = [int(order[2 * N_CORES - 1 - c]), int(order[c])]
        for j in range(EPC):
            e = experts[j]
            toks, ranks, wts = per_e[e]
            n_dev = min(len(toks), caps[j])
            xgc[j, :, :n_dev] = xf[toks[:n_dev]].T.astype(BF16)
            core_toks.append((toks, ranks, wts, n_dev, e))
        tok_lists.append(core_toks)
        in_maps.append(
            {
                "xg": xgc,
                "gww": wg[experts].astype(BF16),
                "uww": wu[experts].astype(BF16),
                "wdp": wd[experts].astype(BF16),
            }
        )
    return in_maps, tok_lists, caps


def kernel(x, gate_w, wg, wu, wd):
    in_dtype = x.dtype
    xf = np.ascontiguousarray(np.asarray(x).reshape(T, H), dtype=np.float32)
    gate_w = np.asarray(gate_w, dtype=np.float32)
    wg = np.asarray(wg, dtype=np.float32)
    wu = np.asarray(wu, dtype=np.float32)
    wd = np.asarray(wd, dtype=np.float32)

    in_maps, tok_lists, caps = _pack(xf, gate_w, wg, wu, wd)
    if caps not in _CACHE:
        _CACHE[caps] = _build_nc(caps)
    nc = _CACHE[caps]
    out1 = np.zeros((T, H), np.float32)
    out2 = np.zeros((T, H), np.float32)

    res = run_bass_kernel_spmd(nc, in_maps, core_ids=list(range(N_CORES)))
    _CACHE["last_in_maps"] = in_maps
    _CACHE["last_caps"] = caps
    _CACHE["nc"] = nc

    for c in range(N_CORES):
        yc = res.results[c]["y"]                           # [EPC, H, C] bf16
        for j in range(EPC):
            toks, ranks, wts, n_dev, e = tok_lists[c][j]
            # raw expert output; combine weight applied here in fp32
            yr = yc[j, :, :n_dev].T.astype(np.float32) * wts[:n_dev, None]
            sel1 = ranks[:n_dev] == 0
            sel2 = ~sel1
            out1[toks[:n_dev][sel1]] = yr[sel1]
            out2[toks[:n_dev][sel2]] = yr[sel2]
            if len(toks) > n_dev:                          # capacity overflow
                extra = toks[n_dev:]
                yextra = _expert_np(xf[extra], wts[n_dev:], wg[e], wu[e], wd[e])
                r = ranks[n_dev:]
                out1[extra[r == 0]] = yextra[r == 0]
                out2[extra[r == 1]] = yextra[r == 1]

    out = (out1 + out2).reshape(B, S, H)
    return out.astype(in_dtype, copy=False)

